# revision 29
# baseline (speedup 1.0000x reference)
"""Trainium2 Bass kernel for NeighborMLPConvLayerLinear (gnn_message_passing).

Strategy (8 NeuronCores, SPMD):
  - Edges (E=1.6M) sharded by output segment: core c owns segments
    [c*6250, (c+1)*6250) = 200k edges (+4800 pad slots). Segments are uniform
    (row_splits = arange*32), so the segment reduction is a stride-32 sum.
  - Gather: single int16 index stream over 256B "pair tokens". Token t packs
    rows (2t, 2t+1) of the [x_in | in_features] table as
    [row_even (128B) | row_odd - row_even (128B)] (delta encoding).
    dma_gather (SBUF-source, transpose=True) delivers token channels on
    partitions: p 0:64 = even row [x|F], p 64:128 = delta. A per-edge 0/1
    parity mask (broadcast-DMA'd from DRAM) reconstructs the edge's row on
    DVE after a DMA partition-shift of the delta half (DVE lanes are
    partition-locked; SBUF+SBUF DVE ops need equal base partitions).
    Token 0 = zeros for pad slots (mask 0). One index per edge halves Q7
    SWDGE descriptor generation (~7.7ns/idx, the bottleneck) vs a dual
    lo/hi zero-row stream scheme.
  - MLP: p1 = W1a^T gm + W1b^T x_out[seg] (stride-0 broadcast rhs) on PE;
    h = GELU(p1 + b1) on Scalar; p2 = W2aug^T [h; 1] on PE;
    eo = p2 * F_gm; segment sums via grouped tensor_reduce on DVE.
"""
import sys

sys.path.insert(0, "/opt/trn_rl_repo")

import numpy as np
import ml_dtypes

from concourse import bacc, bass, mybir, tile
from concourse import bass_utils

BF16 = mybir.dt.bfloat16
F32 = mybir.dt.float32
I16 = mybir.dt.int16

N = 50000
M = 50000
DEG = 32
C_IN = 32
HID = 64
C_OUT = 32

NCORES = 8
SEG_PER_CORE = M // NCORES            # 6250
E_PER_CORE = SEG_PER_CORE * DEG       # 200000
CH = 8192                             # edges per chunk
NCHUNK = 25                           # 204800 slots
SLOTS = NCHUNK * CH
SEG_PAD = SLOTS // DEG                # 6400 segments incl. padding
SEG_PER_CHUNK = CH // DEG             # 256
PSUM_CH = 1024                        # edges per psum tile
KSUB = CH // PSUM_CH                  # 8

TAIL_CH = 4096                        # gathered slots in the tail chunk
TAIL_KSUB = 4                         # psum blocks in the tail chunk

NTOK = 25001                          # zero token + 25000 row pairs
RANKS = (NTOK + 127) // 128           # 196
TOK = RANKS * 128

_NC_CACHE = {}


def build_nc():
    if "nc" in _NC_CACHE:
        return _NC_CACHE["nc"]
    nc = bacc.Bacc("TRN2", target_bir_lowering=False, debug=False,
                   num_devices=NCORES)

    tbl = nc.dram_tensor("tbl", [128, RANKS * 128], BF16, kind="ExternalInput").ap()
    wtbl = nc.dram_tensor("wtbl", [128, 128], BF16, kind="ExternalInput").ap()
    widx = nc.dram_tensor("widx", [128, 8], I16, kind="ExternalInput").ap()
    idx = nc.dram_tensor("idx", [NCHUNK, 128, CH // 16], I16, kind="ExternalInput").ap()
    mmd = nc.dram_tensor("mmd", [NCHUNK, CH], BF16, kind="ExternalInput").ap()
    xo = nc.dram_tensor("xo", [C_IN, SEG_PAD], BF16, kind="ExternalInput").ap()
    wx = nc.dram_tensor("wx", [64, HID], BF16, kind="ExternalInput").ap()
    w1b = nc.dram_tensor("w1b", [C_IN, HID], BF16, kind="ExternalInput").ap()
    w2 = nc.dram_tensor("w2", [HID + 1, C_OUT], BF16, kind="ExternalInput").ap()
    b1 = nc.dram_tensor("b1", [HID, 1], F32, kind="ExternalInput").ap()
    out = nc.dram_tensor("out", [C_OUT, SEG_PAD], F32, kind="ExternalOutput").ap()

    with tile.TileContext(nc) as tc:
        with (
            tc.tile_pool(name="tbl", bufs=1) as tblp,
            tc.tile_pool(name="w", bufs=1) as wp,
            tc.tile_pool(name="idx", bufs=2) as idxp,
            tc.tile_pool(name="mm", bufs=2) as mmp,
            tc.tile_pool(name="tmp", bufs=2) as tmpp,
            tc.tile_pool(name="g", bufs=2) as gp,
            tc.tile_pool(name="h", bufs=1) as hp,
            tc.tile_pool(name="eo", bufs=2) as eop,
            tc.tile_pool(name="red", bufs=2) as redp,
            tc.tile_pool(name="ps1", bufs=2, space="PSUM") as ps1,
            tc.tile_pool(name="ps2", bufs=2, space="PSUM") as ps2,
        ):
            # warmup: a tiny gather against a 256B-token dummy table makes
            # the Q7 load its SWDGE ucode library during the big table DMA
            # instead of serially before the first real gather.
            sb_wt = wp.tile([128, 128], BF16, tag="wt")
            nc.sync.dma_start(out=sb_wt[:], in_=wtbl[:])
            sb_wi = wp.tile([128, 8], I16, tag="wi")
            nc.sync.dma_start(out=sb_wi[:], in_=widx[:])
            # chunk 0's idx/mask loads issue before the table DMA so their
            # completion doesn't queue behind the 6.4MB table transfer.
            it0 = idxp.tile([128, CH // 16], I16, tag="i")
            nc.sync.dma_start(out=it0[:], in_=idx[0])
            mm0 = mmp.tile([64, CH], BF16, tag="mm")
            nc.sync.dma_start(
                out=mm0[:], in_=mmd[0].unsqueeze(0).to_broadcast([64, CH]))
            sb_xo = wp.tile([C_IN, SEG_PAD], BF16, tag="xo")
            nc.sync.dma_start(out=sb_xo[:], in_=xo[:])
            sb_wx = wp.tile([64, HID], BF16, tag="wx")
            nc.sync.dma_start(out=sb_wx[:], in_=wx[:])
            sb_w1b = wp.tile([C_IN, HID], BF16, tag="w1b")
            nc.sync.dma_start(out=sb_w1b[:], in_=w1b[:])
            sb_w2 = wp.tile([HID + 1, C_OUT], BF16, tag="w2")
            nc.sync.dma_start(out=sb_w2[:], in_=w2[:])
            sb_b1 = wp.tile([HID, 1], F32, tag="b1")
            nc.sync.dma_start(out=sb_b1[:], in_=b1[:])
            # the 6.4MB table DMA is issued LAST so every other completion
            # semaphore has already fired when it finishes (DMA waits are
            # gated by cumulative completion counts).
            sb_tbl = tblp.tile([128, RANKS * 128], BF16, tag="tbl")
            nc.sync.dma_start(out=sb_tbl[:], in_=tbl[:])
            wg = wp.tile([128, 128], BF16, tag="wg")
            nc.gpsimd.dma_gather(
                out_ap=wg[:].unsqueeze(1), in_ap=sb_wt[:], idxs_ap=sb_wi[:],
                num_idxs=128, num_idxs_reg=128, elem_size=128,
                transpose=True, sbuf_tokens_per_rank=128,
                sbuf_free_dim_per_rank=256, single_packet=False,
            )

            # h staging: [HID+1, 2*PSUM_CH]; row HID stays 1.0 (bias-via-matmul)
            h_all = hp.tile([HID + 1, 2 * PSUM_CH], BF16, tag="h")
            nc.vector.memset(h_all[HID:HID + 1, :], 1.0)

            for t in range(NCHUNK):
                if t == 0:
                    it, mm = it0, mm0
                else:
                    it = idxp.tile([128, CH // 16], I16, tag="i")
                    nc.sync.dma_start(out=it[:], in_=idx[t])
                    mm = mmp.tile([64, CH], BF16, tag="mm")
                    nc.sync.dma_start(
                        out=mm[:], in_=mmd[t].unsqueeze(0).to_broadcast([64, CH]))

                g = gp.tile([128, CH], BF16, tag="g")
                # tail chunk holds only 3392 real edges (segments 6144..6250);
                # gather and compute only what is real (rounded to 1024). The
                # tail runs in 2048-halves (gather + select per half) so the
                # first half's MLP hides under the second half's desc-gen.
                is_tail = t == NCHUNK - 1
                tch = TAIL_CH if is_tail else CH
                ksub_t = TAIL_KSUB if is_tail else KSUB
                # tail runs in 2048-halves so the first half's MLP hides
                # under the second half's desc-gen
                bounds = ([0, 2048, 4096] if is_tail
                          else [0, CH // 2, CH])
                tmp = tmpp.tile([64, CH], BF16, tag="tmp")
                for gi in range(len(bounds) - 1):
                    lo, hi = bounds[gi], bounds[gi + 1]
                    GH = hi - lo
                    nc.gpsimd.dma_gather(
                        out_ap=g[:, lo:hi].unsqueeze(1),
                        in_ap=sb_tbl[:],
                        idxs_ap=it[:, lo // 16:hi // 16],
                        num_idxs=GH, num_idxs_reg=GH, elem_size=128,
                        transpose=True, sbuf_tokens_per_rank=128,
                        sbuf_free_dim_per_rank=256, single_packet=False,
                    )
                    # row select: g[0:64] += parity * g_delta. The delta half
                    # is DMA-shifted to base partition 0 first.
                    nc.sync.dma_start(out=tmp[:, lo:hi], in_=g[64:128, lo:hi])
                    nc.vector.tensor_tensor(out=tmp[:, lo:hi],
                                            in0=tmp[:, lo:hi],
                                            in1=mm[:, lo:hi],
                                            op=mybir.AluOpType.mult)
                    nc.vector.tensor_tensor(out=g[0:64, lo:hi],
                                            in0=g[0:64, lo:hi],
                                            in1=tmp[:, lo:hi],
                                            op=mybir.AluOpType.add)
                gm = g

                red = redp.tile([C_OUT, SEG_PER_CHUNK], F32, tag="red")
                for k in range(ksub_t):
                    e0 = k * PSUM_CH
                    p1 = ps1.tile([HID, PSUM_CH], F32, tag="p1")
                    for j in range(PSUM_CH // 512):
                        c0 = e0 + j * 512
                        s0 = (t * CH + c0) // DEG
                        nc.tensor.matmul(out=p1[:, j * 512:(j + 1) * 512],
                                         lhsT=sb_wx[:], rhs=gm[0:64, c0:c0 + 512],
                                         start=True, stop=False)
                        xo_b = sb_xo[:, s0:s0 + 16].unsqueeze(2).to_broadcast(
                            [C_IN, 16, DEG])
                        nc.tensor.matmul(out=p1[:, j * 512:(j + 1) * 512],
                                         lhsT=sb_w1b[:], rhs=xo_b,
                                         start=False, stop=True)
                    hs = h_all[:, (k % 2) * PSUM_CH:(k % 2 + 1) * PSUM_CH]
                    nc.scalar.activation(hs[0:HID, :], p1[:],
                                         mybir.ActivationFunctionType.Gelu,
                                         bias=sb_b1[:], scale=1.0)
                    p2 = ps2.tile([C_OUT, PSUM_CH], F32, tag="p2")
                    for j in range(PSUM_CH // 512):
                        nc.tensor.matmul(out=p2[:, j * 512:(j + 1) * 512],
                                         lhsT=sb_w2[:],
                                         rhs=hs[:, j * 512:(j + 1) * 512],
                                         start=True, stop=True)
                    eo = eop.tile([C_OUT, PSUM_CH], BF16, tag="eo")
                    nc.vector.tensor_tensor(out=eo[:], in0=p2[:],
                                            in1=gm[C_IN:64, e0:e0 + PSUM_CH],
                                            op=mybir.AluOpType.mult)
                    nc.vector.tensor_reduce(
                        out=red[:, k * (PSUM_CH // DEG):(k + 1) * (PSUM_CH // DEG)],
                        in_=eo[:].rearrange("p (s e) -> p s e", e=DEG),
                        axis=mybir.AxisListType.X, op=mybir.AluOpType.add)
                segs_t = ksub_t * (PSUM_CH // DEG)
                nc.sync.dma_start(
                    out=out[:, t * SEG_PER_CHUNK:t * SEG_PER_CHUNK + segs_t],
                    in_=red[:, 0:segs_t])
    nc.compile()
    _NC_CACHE["nc"] = nc
    return nc


def _wrap(a):
    """slot i -> partition i%16, col i//16; replicated over 8 groups."""
    w = a.reshape(NCHUNK, CH // 16, 16).transpose(0, 2, 1)
    return np.tile(w, (1, 8, 1)).copy()


def prep_in_maps(x_in, x_out, in_features, neighbors_index, neighbors_row_splits,
                 W1, b1, W2, b2):
    x_in = np.asarray(x_in, np.float32)
    x_out = np.asarray(x_out, np.float32)
    in_features = np.asarray(in_features, np.float32)
    idx = np.asarray(neighbors_index, np.int64)
    W1 = np.asarray(W1, np.float32)
    b1v = np.asarray(b1, np.float32)
    W2 = np.asarray(W2, np.float32)
    b2v = np.asarray(b2, np.float32)

    rows = np.empty((N, 64), np.float32)
    rows[:, 0:C_IN] = x_in
    rows[:, C_IN:64] = in_features
    rows16 = rows.astype(ml_dtypes.bfloat16)

    # pair-token table: token 0 = zeros; token t>=1 = rows (2t-2, 2t-1) as
    # [row_even | row_odd - row_even]
    toks = np.zeros((TOK, 128), dtype=ml_dtypes.bfloat16)
    toks[1:N // 2 + 1, 0:64] = rows16[0::2]
    toks[1:N // 2 + 1, 64:128] = (
        rows16[1::2].astype(np.float32) - rows16[0::2].astype(np.float32)
    ).astype(ml_dtypes.bfloat16)
    tbl = toks.reshape(RANKS, 128, 128).transpose(1, 0, 2).reshape(
        128, RANKS * 128).copy()

    wx = np.zeros((64, HID), dtype=ml_dtypes.bfloat16)
    wx[0:C_IN] = W1[0:C_IN].astype(ml_dtypes.bfloat16)
    w1b = W1[C_IN:].astype(ml_dtypes.bfloat16)
    w2aug = np.zeros((HID + 1, C_OUT), dtype=ml_dtypes.bfloat16)
    w2aug[0:HID] = (W2 / DEG).astype(ml_dtypes.bfloat16)
    w2aug[HID] = (b2v / DEG).astype(ml_dtypes.bfloat16)
    b1c = b1v.reshape(HID, 1).copy()

    in_maps = []
    pad = SLOTS - E_PER_CORE
    for c in range(NCORES):
        v = np.concatenate([idx[c * E_PER_CORE:(c + 1) * E_PER_CORE],
                            np.full(pad, -1, np.int64)])
        tok = np.where(v < 0, 0, (v >> 1) + 1).astype(np.int16)
        par = np.where(v < 0, 0.0, (v & 1).astype(np.float32))
        mmc = par.astype(ml_dtypes.bfloat16).reshape(NCHUNK, CH)
        xoc = np.zeros((C_IN, SEG_PAD), dtype=ml_dtypes.bfloat16)
        xoc[:, :SEG_PER_CORE] = x_out[
            c * SEG_PER_CORE:(c + 1) * SEG_PER_CORE].T.astype(ml_dtypes.bfloat16)
        in_maps.append({
            "tbl": tbl, "idx": _wrap(tok), "mmd": mmc, "xo": xoc,
            "wx": wx, "w1b": w1b, "w2": w2aug, "b1": b1c,
            "wtbl": np.zeros((128, 128), dtype=ml_dtypes.bfloat16),
            "widx": np.zeros((128, 8), np.int16),
        })
    return in_maps


def kernel(**inputs):
    in_maps = prep_in_maps(**inputs)
    global _LAST_IN_MAPS
    _LAST_IN_MAPS = in_maps
    nc = build_nc()
    res = bass_utils.run_bass_kernel_spmd(nc, in_maps, list(range(NCORES))).results
    out = np.empty((M, C_OUT), np.float32)
    for c in range(NCORES):
        out[c * SEG_PER_CORE:(c + 1) * SEG_PER_CORE] = \
            res[c]["out"][:, :SEG_PER_CORE].T
    return out


# revision 30
# speedup vs baseline: 1.0039x; 1.0039x over previous
"""Trainium2 Bass kernel for NeighborMLPConvLayerLinear (gnn_message_passing).

Strategy (8 NeuronCores, SPMD):
  - Edges (E=1.6M) sharded by output segment: core c owns segments
    [c*6250, (c+1)*6250) = 200k edges (+4800 pad slots). Segments are uniform
    (row_splits = arange*32), so the segment reduction is a stride-32 sum.
  - Gather: single int16 index stream over 256B "pair tokens". Token t packs
    rows (2t, 2t+1) of the [x_in | in_features] table as
    [row_even (128B) | row_odd - row_even (128B)] (delta encoding).
    dma_gather (SBUF-source, transpose=True) delivers token channels on
    partitions: p 0:64 = even row [x|F], p 64:128 = delta. A per-edge 0/1
    parity mask (broadcast-DMA'd from DRAM) reconstructs the edge's row on
    DVE after a DMA partition-shift of the delta half (DVE lanes are
    partition-locked; SBUF+SBUF DVE ops need equal base partitions).
    Token 0 = zeros for pad slots (mask 0). One index per edge halves Q7
    SWDGE descriptor generation (~7.7ns/idx, the bottleneck) vs a dual
    lo/hi zero-row stream scheme.
  - MLP: p1 = W1a^T gm + W1b^T x_out[seg] (stride-0 broadcast rhs) on PE;
    h = GELU(p1 + b1) on Scalar; p2 = W2aug^T [h; 1] on PE;
    eo = p2 * F_gm; segment sums via grouped tensor_reduce on DVE.
"""
import sys

sys.path.insert(0, "/opt/trn_rl_repo")

import numpy as np
import ml_dtypes

from concourse import bacc, bass, mybir, tile
from concourse import bass_utils

BF16 = mybir.dt.bfloat16
F32 = mybir.dt.float32
I16 = mybir.dt.int16

N = 50000
M = 50000
DEG = 32
C_IN = 32
HID = 64
C_OUT = 32

NCORES = 8
SEG_PER_CORE = M // NCORES            # 6250
E_PER_CORE = SEG_PER_CORE * DEG       # 200000
CH = 8192                             # edges per chunk
NCHUNK = 25                           # 204800 slots
SLOTS = NCHUNK * CH
SEG_PAD = SLOTS // DEG                # 6400 segments incl. padding
SEG_PER_CHUNK = CH // DEG             # 256
PSUM_CH = 1024                        # edges per psum tile
KSUB = CH // PSUM_CH                  # 8

TAIL_CH = 4096                        # gathered slots in the tail chunk
TAIL_KSUB = 4                         # psum blocks in the tail chunk

NTOK = 25001                          # zero token + 25000 row pairs
RANKS = (NTOK + 127) // 128           # 196
TOK = RANKS * 128

_NC_CACHE = {}


def build_nc():
    if "nc" in _NC_CACHE:
        return _NC_CACHE["nc"]
    nc = bacc.Bacc("TRN2", target_bir_lowering=False, debug=False,
                   num_devices=NCORES)

    tbl = nc.dram_tensor("tbl", [128, RANKS * 128], BF16, kind="ExternalInput").ap()
    wtbl = nc.dram_tensor("wtbl", [128, 128], BF16, kind="ExternalInput").ap()
    widx = nc.dram_tensor("widx", [128, 8], I16, kind="ExternalInput").ap()
    idx = nc.dram_tensor("idx", [NCHUNK, 128, CH // 16], I16, kind="ExternalInput").ap()
    mmd = nc.dram_tensor("mmd", [NCHUNK, CH], BF16, kind="ExternalInput").ap()
    xo = nc.dram_tensor("xo", [C_IN, SEG_PAD], BF16, kind="ExternalInput").ap()
    wx = nc.dram_tensor("wx", [64, HID], BF16, kind="ExternalInput").ap()
    w1b = nc.dram_tensor("w1b", [C_IN, HID], BF16, kind="ExternalInput").ap()
    w2 = nc.dram_tensor("w2", [HID + 1, C_OUT], BF16, kind="ExternalInput").ap()
    b1 = nc.dram_tensor("b1", [HID, 1], F32, kind="ExternalInput").ap()
    out = nc.dram_tensor("out", [C_OUT, SEG_PAD], F32, kind="ExternalOutput").ap()

    with tile.TileContext(nc) as tc:
        with (
            tc.tile_pool(name="tbl", bufs=1) as tblp,
            tc.tile_pool(name="w", bufs=1) as wp,
            tc.tile_pool(name="idx", bufs=2) as idxp,
            tc.tile_pool(name="mm", bufs=2) as mmp,
            tc.tile_pool(name="tmp", bufs=2) as tmpp,
            tc.tile_pool(name="g", bufs=2) as gp,
            tc.tile_pool(name="h", bufs=1) as hp,
            tc.tile_pool(name="eo", bufs=2) as eop,
            tc.tile_pool(name="red", bufs=2) as redp,
            tc.tile_pool(name="ps1", bufs=2, space="PSUM") as ps1,
            tc.tile_pool(name="ps2", bufs=2, space="PSUM") as ps2,
        ):
            # warmup: a tiny gather against a 256B-token dummy table makes
            # the Q7 load its SWDGE ucode library during the big table DMA
            # instead of serially before the first real gather.
            sb_wt = wp.tile([128, 128], BF16, tag="wt")
            nc.sync.dma_start(out=sb_wt[:], in_=wtbl[:])
            sb_wi = wp.tile([128, 8], I16, tag="wi")
            nc.sync.dma_start(out=sb_wi[:], in_=widx[:])
            # chunk 0's idx/mask loads issue before the table DMA so their
            # completion doesn't queue behind the 6.4MB table transfer.
            it0 = idxp.tile([128, CH // 16], I16, tag="i")
            nc.sync.dma_start(out=it0[:], in_=idx[0])
            mm0 = mmp.tile([64, CH], BF16, tag="mm")
            nc.sync.dma_start(
                out=mm0[:], in_=mmd[0].unsqueeze(0).to_broadcast([64, CH]))
            sb_tbl = tblp.tile([128, RANKS * 128], BF16, tag="tbl")
            nc.sync.dma_start(out=sb_tbl[:], in_=tbl[:])
            wg = wp.tile([128, 128], BF16, tag="wg")
            nc.gpsimd.dma_gather(
                out_ap=wg[:].unsqueeze(1), in_ap=sb_wt[:], idxs_ap=sb_wi[:],
                num_idxs=128, num_idxs_reg=128, elem_size=128,
                transpose=True, sbuf_tokens_per_rank=128,
                sbuf_free_dim_per_rank=256, single_packet=False,
            )

            sb_xo = wp.tile([C_IN, SEG_PAD], BF16, tag="xo")
            nc.sync.dma_start(out=sb_xo[:], in_=xo[:])
            sb_wx = wp.tile([64, HID], BF16, tag="wx")
            nc.sync.dma_start(out=sb_wx[:], in_=wx[:])
            sb_w1b = wp.tile([C_IN, HID], BF16, tag="w1b")
            nc.sync.dma_start(out=sb_w1b[:], in_=w1b[:])
            sb_w2 = wp.tile([HID + 1, C_OUT], BF16, tag="w2")
            nc.sync.dma_start(out=sb_w2[:], in_=w2[:])
            sb_b1 = wp.tile([HID, 1], F32, tag="b1")
            nc.sync.dma_start(out=sb_b1[:], in_=b1[:])

            # h staging: [HID+1, 2*PSUM_CH]; row HID stays 1.0 (bias-via-matmul)
            h_all = hp.tile([HID + 1, 2 * PSUM_CH], BF16, tag="h")
            nc.vector.memset(h_all[HID:HID + 1, :], 1.0)

            for t in range(NCHUNK):
                if t == 0:
                    it, mm = it0, mm0
                else:
                    it = idxp.tile([128, CH // 16], I16, tag="i")
                    nc.sync.dma_start(out=it[:], in_=idx[t])
                    mm = mmp.tile([64, CH], BF16, tag="mm")
                    nc.sync.dma_start(
                        out=mm[:], in_=mmd[t].unsqueeze(0).to_broadcast([64, CH]))

                g = gp.tile([128, CH], BF16, tag="g")
                # tail chunk holds only 3392 real edges (segments 6144..6250);
                # gather and compute only what is real (rounded to 1024). The
                # tail runs in 2048-halves (gather + select per half) so the
                # first half's MLP hides under the second half's desc-gen.
                is_tail = t == NCHUNK - 1
                tch = TAIL_CH if is_tail else CH
                ksub_t = TAIL_KSUB if is_tail else KSUB
                # tail runs in 2048-halves so the first half's MLP hides
                # under the second half's desc-gen
                bounds = ([0, 2048, 4096] if is_tail
                          else [0, CH // 2, CH])
                tmp = tmpp.tile([64, CH], BF16, tag="tmp")
                for gi in range(len(bounds) - 1):
                    lo, hi = bounds[gi], bounds[gi + 1]
                    GH = hi - lo
                    nc.gpsimd.dma_gather(
                        out_ap=g[:, lo:hi].unsqueeze(1),
                        in_ap=sb_tbl[:],
                        idxs_ap=it[:, lo // 16:hi // 16],
                        num_idxs=GH, num_idxs_reg=GH, elem_size=128,
                        transpose=True, sbuf_tokens_per_rank=128,
                        sbuf_free_dim_per_rank=256, single_packet=False,
                    )
                    # row select: g[0:64] += parity * g_delta. The delta half
                    # is DMA-shifted to base partition 0 first.
                    nc.sync.dma_start(out=tmp[:, lo:hi], in_=g[64:128, lo:hi])
                    nc.vector.tensor_tensor(out=tmp[:, lo:hi],
                                            in0=tmp[:, lo:hi],
                                            in1=mm[:, lo:hi],
                                            op=mybir.AluOpType.mult)
                    nc.vector.tensor_tensor(out=g[0:64, lo:hi],
                                            in0=g[0:64, lo:hi],
                                            in1=tmp[:, lo:hi],
                                            op=mybir.AluOpType.add)
                gm = g

                red = redp.tile([C_OUT, SEG_PER_CHUNK], F32, tag="red")
                for k in range(ksub_t):
                    e0 = k * PSUM_CH
                    p1 = ps1.tile([HID, PSUM_CH], F32, tag="p1")
                    for j in range(PSUM_CH // 512):
                        c0 = e0 + j * 512
                        s0 = (t * CH + c0) // DEG
                        nc.tensor.matmul(out=p1[:, j * 512:(j + 1) * 512],
                                         lhsT=sb_wx[:], rhs=gm[0:64, c0:c0 + 512],
                                         start=True, stop=False)
                        xo_b = sb_xo[:, s0:s0 + 16].unsqueeze(2).to_broadcast(
                            [C_IN, 16, DEG])
                        nc.tensor.matmul(out=p1[:, j * 512:(j + 1) * 512],
                                         lhsT=sb_w1b[:], rhs=xo_b,
                                         start=False, stop=True)
                    hs = h_all[:, (k % 2) * PSUM_CH:(k % 2 + 1) * PSUM_CH]
                    nc.scalar.activation(hs[0:HID, :], p1[:],
                                         mybir.ActivationFunctionType.Gelu,
                                         bias=sb_b1[:], scale=1.0)
                    p2 = ps2.tile([C_OUT, PSUM_CH], F32, tag="p2")
                    for j in range(PSUM_CH // 512):
                        nc.tensor.matmul(out=p2[:, j * 512:(j + 1) * 512],
                                         lhsT=sb_w2[:],
                                         rhs=hs[:, j * 512:(j + 1) * 512],
                                         start=True, stop=True)
                    eo = eop.tile([C_OUT, PSUM_CH], BF16, tag="eo")
                    nc.vector.tensor_tensor(out=eo[:], in0=p2[:],
                                            in1=gm[C_IN:64, e0:e0 + PSUM_CH],
                                            op=mybir.AluOpType.mult)
                    nc.vector.tensor_reduce(
                        out=red[:, k * (PSUM_CH // DEG):(k + 1) * (PSUM_CH // DEG)],
                        in_=eo[:].rearrange("p (s e) -> p s e", e=DEG),
                        axis=mybir.AxisListType.X, op=mybir.AluOpType.add)
                segs_t = ksub_t * (PSUM_CH // DEG)
                nc.sync.dma_start(
                    out=out[:, t * SEG_PER_CHUNK:t * SEG_PER_CHUNK + segs_t],
                    in_=red[:, 0:segs_t])
    nc.compile()
    _NC_CACHE["nc"] = nc
    return nc


def _wrap(a):
    """slot i -> partition i%16, col i//16; replicated over 8 groups."""
    w = a.reshape(NCHUNK, CH // 16, 16).transpose(0, 2, 1)
    return np.tile(w, (1, 8, 1)).copy()


def prep_in_maps(x_in, x_out, in_features, neighbors_index, neighbors_row_splits,
                 W1, b1, W2, b2):
    x_in = np.asarray(x_in, np.float32)
    x_out = np.asarray(x_out, np.float32)
    in_features = np.asarray(in_features, np.float32)
    idx = np.asarray(neighbors_index, np.int64)
    W1 = np.asarray(W1, np.float32)
    b1v = np.asarray(b1, np.float32)
    W2 = np.asarray(W2, np.float32)
    b2v = np.asarray(b2, np.float32)

    rows = np.empty((N, 64), np.float32)
    rows[:, 0:C_IN] = x_in
    rows[:, C_IN:64] = in_features
    rows16 = rows.astype(ml_dtypes.bfloat16)

    # pair-token table: token 0 = zeros; token t>=1 = rows (2t-2, 2t-1) as
    # [row_even | row_odd - row_even]
    toks = np.zeros((TOK, 128), dtype=ml_dtypes.bfloat16)
    toks[1:N // 2 + 1, 0:64] = rows16[0::2]
    toks[1:N // 2 + 1, 64:128] = (
        rows16[1::2].astype(np.float32) - rows16[0::2].astype(np.float32)
    ).astype(ml_dtypes.bfloat16)
    tbl = toks.reshape(RANKS, 128, 128).transpose(1, 0, 2).reshape(
        128, RANKS * 128).copy()

    wx = np.zeros((64, HID), dtype=ml_dtypes.bfloat16)
    wx[0:C_IN] = W1[0:C_IN].astype(ml_dtypes.bfloat16)
    w1b = W1[C_IN:].astype(ml_dtypes.bfloat16)
    w2aug = np.zeros((HID + 1, C_OUT), dtype=ml_dtypes.bfloat16)
    w2aug[0:HID] = (W2 / DEG).astype(ml_dtypes.bfloat16)
    w2aug[HID] = (b2v / DEG).astype(ml_dtypes.bfloat16)
    b1c = b1v.reshape(HID, 1).copy()

    in_maps = []
    pad = SLOTS - E_PER_CORE
    for c in range(NCORES):
        v = np.concatenate([idx[c * E_PER_CORE:(c + 1) * E_PER_CORE],
                            np.full(pad, -1, np.int64)])
        tok = np.where(v < 0, 0, (v >> 1) + 1).astype(np.int16)
        par = np.where(v < 0, 0.0, (v & 1).astype(np.float32))
        mmc = par.astype(ml_dtypes.bfloat16).reshape(NCHUNK, CH)
        xoc = np.zeros((C_IN, SEG_PAD), dtype=ml_dtypes.bfloat16)
        xoc[:, :SEG_PER_CORE] = x_out[
            c * SEG_PER_CORE:(c + 1) * SEG_PER_CORE].T.astype(ml_dtypes.bfloat16)
        in_maps.append({
            "tbl": tbl, "idx": _wrap(tok), "mmd": mmc, "xo": xoc,
            "wx": wx, "w1b": w1b, "w2": w2aug, "b1": b1c,
            "wtbl": np.zeros((128, 128), dtype=ml_dtypes.bfloat16),
            "widx": np.zeros((128, 8), np.int16),
        })
    return in_maps


def kernel(**inputs):
    in_maps = prep_in_maps(**inputs)
    global _LAST_IN_MAPS
    _LAST_IN_MAPS = in_maps
    nc = build_nc()
    res = bass_utils.run_bass_kernel_spmd(nc, in_maps, list(range(NCORES))).results
    out = np.empty((M, C_OUT), np.float32)
    for c in range(NCORES):
        out[c * SEG_PER_CORE:(c + 1) * SEG_PER_CORE] = \
            res[c]["out"][:, :SEG_PER_CORE].T
    return out
